# revision 47
# baseline (speedup 1.0000x reference)
"""BiViewMixHop GNN kernel for 8 Trainium2 NeuronCores (Bass/Tile).

Strategy (v2):
  - Algebraic restructure: P(h)@W1 + P^2(h)@W2 = P(h@W1 + P(h@W2)); hom/het
    views fused into one 128-col tensor -> 2 gather passes per layer (6 total).
  - Host prep (index manipulation only): relabel nodes into graph-aligned
    32-slot-padded "slots", shard whole graphs contiguously across 8 cores,
    sort each core's edges by (dst group, src half), pad each (group, half)
    edge list to a multiple of 128 ("chunks"). Chunk counts maxed across
    cores so ONE SPMD program serves all 8 cores.
  - Gather: bulk SWDGE dma_gather (InstDMAGatherAnt) - ONE instruction per
    16-chunk slice (2048 rows) instead of one indirect DMA per chunk;
    SWDGE cost is ~1us fixed + 0.34ns/row, so this is ~15x less gpsimd
    time than per-chunk indirect DMAs.  int16 gather indices only address
    32767 rows, so the AllGathered node table is split in two halves at
    the core-4 boundary and chunks are (dst-group, src-half)-pure.
  - bf16 data plane: node tables, gathered tiles, selection matrices and
    weights in bf16 (PE matmuls 4x faster than f32; half the HBM/collective
    bytes). PSUM accumulation stays f32.
  - Scatter/segment-sum: selection matrices S[e,d] = (dstloc==d)*mask built
    on DVE, PE matmuls accumulate masked messages into per-group PSUM.
  - Degree normalization: ones-vector matmuls folded into the layer-0
    pass-A sweep (reuses the S matrices; no separate prologue); 1/deg
    applied as per-partition ACT scale at PSUM evacuation.
  - Cross-core exchange of gather-source tables via DRAM AllGather (bf16).
  - Readout: mean-pool via PE matmuls against one-hot graph selectors;
    max-pool via windowed reduce + per-graph mask+reduce; AllReduce across
    cores; small MLP head + log_softmax replicated per core.
"""

import numpy as np

# ---------------------------------------------------------------- constants
F_IN = 128
H_HID = 64
N_LAYERS = 3
N_GRAPHS = 256
N_CLS = 10
NCORES = 8
P = 128
DGRP = 32           # dst-group width (selection-matrix columns)
SBATCH = 16         # chunks per S-build batch
KG = 8              # chunks per dma_gather slice
W_G = 64            # per-core local-graph window
NQ = 4              # SWDGE queues
NGP = N_GRAPHS + 8  # pooled-buffer rows (graphs + dump row for pads)
BISECT_NO_IDMA = False  # debug: skip final indirect scatters
BISECT_LAYERS = N_LAYERS  # debug: number of layers to run


# ------------------------------------------------------------------ host prep
def _prep(edge_index, batch, hom_mask, het_mask):
    """Index-only preprocessing. Returns structure dict + per-core arrays."""
    N = batch.shape[0]
    src, dst = np.asarray(edge_index[0]), np.asarray(edge_index[1])
    batch = np.asarray(batch)
    counts = np.bincount(batch, minlength=N_GRAPHS)

    pad_sizes = ((counts + DGRP - 1) // DGRP) * DGRP
    total_slots = int(pad_sizes.sum())
    target = max(total_slots / NCORES, 1.0)

    csum = np.concatenate([[0], np.cumsum(pad_sizes)])
    gcore = np.minimum((csum[:-1] + pad_sizes / 2) / target, NCORES - 1).astype(np.int64)
    gcore = np.maximum.accumulate(gcore)

    core_slots = np.zeros(NCORES, np.int64)
    graph_base = np.zeros(N_GRAPHS, np.int64)
    for g in range(N_GRAPHS):
        c = gcore[g]
        graph_base[g] = core_slots[c]
        core_slots[c] += pad_sizes[g]
    S_core = int(((core_slots.max() + P - 1) // P) * P)
    TILES = S_core // P
    GROUPS = S_core // DGRP
    TOT = NCORES * S_core
    HALF = TOT // 2
    assert HALF <= 32767, f"half-table rows {HALF} exceed int16 gather range"

    node_starts = np.concatenate([[0], np.cumsum(counts)])
    node_slot = np.empty(N, np.int64)
    for g in range(N_GRAPHS):
        a, b = node_starts[g], node_starts[g + 1]
        if b > a:
            base = gcore[g] * S_core + graph_base[g]
            node_slot[a:b] = base + np.arange(b - a)

    src_slot = node_slot[src]
    dst_slot = node_slot[dst]
    owner = dst_slot // S_core
    half = (src_slot >= HALF).astype(np.int64)

    core_graphs = [np.where((gcore == c) & (counts > 0))[0] for c in range(NCORES)]
    n_local = max(len(cg) for cg in core_graphs)
    assert n_local <= W_G, f"{n_local} local graphs > W_G={W_G}"

    hom_mask = np.asarray(hom_mask)
    het_mask = np.asarray(het_mask)
    per_core_edges = []
    cntLO = np.zeros((NCORES, TILES), np.int64)
    cntHI = np.zeros((NCORES, TILES), np.int64)
    for c in range(NCORES):
        m = owner == c
        es, ed = src_slot[m], dst_slot[m]
        eh, et, hf = hom_mask[m], het_mask[m], half[m]
        loc = ed - c * S_core
        grp = loc // P
        order = np.lexsort((hf, grp))
        es, loc, eh, et, hf, grp = (a[order] for a in (es, loc, eh, et, hf, grp))
        cnt2 = np.bincount(grp * 2 + hf, minlength=TILES * 2)
        cntLO[c] = cnt2[0::2]
        cntHI[c] = cnt2[1::2]
        per_core_edges.append((es, loc, eh, et, grp, hf))

    cpgLO = np.maximum.reduce((cntLO + P - 1) // P, axis=0)
    cpgHI = np.maximum.reduce((cntHI + P - 1) // P, axis=0)
    both0 = (cpgLO + cpgHI) == 0
    cpgLO[both0] = 1
    C_LO = int(cpgLO.sum())
    C_HI = int(cpgHI.sum())
    C_CH = C_LO + C_HI
    cogLO = np.concatenate([[0], np.cumsum(cpgLO)])
    cogHI = np.concatenate([[0], np.cumsum(cpgHI)])

    st = {"S_core": S_core, "TILES": TILES, "GROUPS": GROUPS, "TOT": TOT,
          "HALF": HALF, "C_LO": C_LO, "C_HI": C_HI, "C_CH": C_CH,
          "cpgLO": cpgLO, "cpgHI": cpgHI, "cogLO": cogLO, "cogHI": cogHI,
          "node_slot": node_slot, "counts": counts}

    def wrap_idx(cols):
        # dma_gather idx layout: idx i at partition i%16, col i//16,
        # replicated across the eight 16-partition bands.
        ncols = cols.shape[1]
        if ncols == 0:
            return np.zeros((P, 8), np.int16)
        flat = cols.T.reshape(-1)
        w = flat.reshape(-1, 16).T
        return np.ascontiguousarray(np.tile(w, (8, 1))).astype(np.int16)

    per_core = []
    for c in range(NCORES):
        es, loc, eh, et, grp, hf = per_core_edges[c]
        # per-(group,half) start offsets in the sorted edge array
        key = grp * 2 + hf
        gstart = np.searchsorted(key, np.arange(TILES * 2))
        gend = np.searchsorted(key, np.arange(TILES * 2) + 1)

        idxA = np.zeros((P, C_CH), np.int16)
        dlA = np.zeros((P, C_CH), np.int64)
        mhA = np.zeros((P, C_CH), np.float32)
        mtA = np.zeros((P, C_CH), np.float32)
        for g in range(TILES):
            for h, (cog, cpg, base) in enumerate(
                    ((cogLO, cpgLO, 0), (cogHI, cpgHI, C_LO))):
                a, b = gstart[2 * g + h], gend[2 * g + h]
                c0 = base + cog[g]
                for j in range(cpg[g]):
                    lo = a + j * P
                    hi = min(a + (j + 1) * P, b)
                    if hi <= lo:
                        break
                    k = hi - lo
                    idxA[:k, c0 + j] = (es[lo:hi] - h * HALF).astype(np.int16)
                    dlA[:k, c0 + j] = loc[lo:hi] - g * P
                    mhA[:k, c0 + j] = eh[lo:hi]
                    mtA[:k, c0 + j] = et[lo:hi]

        batchloc = np.full(S_core, -1.0, np.float32)
        maskrow = np.zeros((W_G, GROUPS), np.float32)
        gidx = np.full((W_G, 1), N_GRAPHS, np.int32)  # pads -> dump row
        for li, g in enumerate(core_graphs[c]):
            base = graph_base[g]
            batchloc[base:base + counts[g]] = li
            g0, g1 = base // DGRP, (base + pad_sizes[g]) // DGRP
            maskrow[li, g0:g1] = 1.0
            gidx[li, 0] = g
        per_core.append({
            "idxLO": wrap_idx(idxA[:, :C_LO]),
            "idxHI": wrap_idx(idxA[:, C_LO:]),
            "dlA": dlA.astype(np.float32), "mhA": mhA, "mtA": mtA,
            "batchloc": batchloc.reshape(TILES, P).T.copy(),
            "maskrow": maskrow, "gidx": gidx})
    return st, per_core


# ------------------------------------------------------------- device builder
def _build(st):
    import concourse.bass as bass
    import concourse.bacc as bacc
    import concourse.mybir as mybir
    import concourse.tile as tile
    from concourse.masks import make_identity

    S_core, TILES, GROUPS = st["S_core"], st["TILES"], st["GROUPS"]
    TOT, HALF = st["TOT"], st["HALF"]
    C_LO, C_HI, C_CH = st["C_LO"], st["C_HI"], st["C_CH"]
    cpgLO, cpgHI = st["cpgLO"], st["cpgHI"]
    cogLO, cogHI = st["cogLO"], st["cogHI"]
    f32 = mybir.dt.float32
    bf16 = mybir.dt.bfloat16
    F2 = 2 * H_HID  # 128

    nc = bacc.Bacc("TRN2", target_bir_lowering=False, debug=False,
                   num_devices=NCORES, num_swdge_queues=NQ)

    def din(name, shape, dtype=f32):
        return nc.dram_tensor(name, shape, dtype, kind="ExternalInput").ap()

    x_own = din("x_own", [S_core, F_IN])
    idxLO = din("idxLO", [P, max(C_LO, 1) * 8], mybir.dt.int16)
    idxHI = din("idxHI", [P, max(C_HI, 1) * 8], mybir.dt.int16)
    dlA = din("dlA", [P, C_CH])
    mhA = din("mhA", [P, C_CH])
    mtA = din("mtA", [P, C_CH])
    batchloc = din("batchloc", [P, TILES])
    maskrow = din("maskrow", [W_G, GROUPS])
    gidx = din("gidx", [W_G, 1], mybir.dt.int32)
    rcount_g = din("rcount_g", [NGP, 1])
    w2cat = din("w2cat", [N_LAYERS, F_IN, F2])
    w1cat = din("w1cat", [N_LAYERS, F_IN, F2])
    w0cat = din("w0cat", [N_LAYERS, F_IN, F2])
    bcat = din("bcat", [N_LAYERS, 1, F2])
    lin1w = din("lin1w", [4 * H_HID, 2 * H_HID])
    lin1b = din("lin1b", [2 * H_HID, 1])
    lin2w = din("lin2w", [2 * H_HID, H_HID])
    lin2b = din("lin2b", [H_HID, 1])
    lin3w = din("lin3w", [H_HID, N_CLS])
    lin3b = din("lin3b", [N_CLS, 1])

    out = nc.dram_tensor("out", [N_GRAPHS, N_CLS], f32, kind="ExternalOutput").ap()

    c_own = nc.dram_tensor("c_own", [S_core, F_IN], bf16).ap()
    u_own = nc.dram_tensor("u_own", [S_core, F_IN], bf16).ap()
    c_full = nc.dram_tensor("c_full", [TOT, F_IN], bf16, addr_space="Shared").ap()
    u_full = nc.dram_tensor("u_full", [TOT, F_IN], bf16, addr_space="Shared").ap()
    maxbuf = nc.dram_tensor("maxbuf", [NGP, F_IN], f32).ap()
    sumbuf = nc.dram_tensor("sumbuf", [NGP, F_IN], f32).ap()
    maxbuf_o = nc.dram_tensor("maxbuf_o", [NGP, F_IN], f32, addr_space="Shared").ap()
    sumbuf_o = nc.dram_tensor("sumbuf_o", [NGP, F_IN], f32, addr_space="Shared").ap()
    mrow_d = nc.dram_tensor("mrow_d", [W_G, GROUPS], bf16).ap()

    with tile.TileContext(nc) as tc:
        with tc.tile_pool(name="const", bufs=1) as cpool, \
             tc.tile_pool(name="sb", bufs=4) as spool, \
             tc.tile_pool(name="gt", bufs=8) as gpool, \
             tc.tile_pool(name="stg", bufs=3) as stgpool, \
             tc.tile_pool(name="sm", bufs=4) as smpool, \
             tc.tile_pool(name="psA", bufs=2, space="PSUM") as psA, \
             tc.tile_pool(name="psB", bufs=2, space="PSUM") as psB, \
             tc.tile_pool(name="psC", bufs=1, space="PSUM") as psC, \
             tc.tile_pool(name="psT", bufs=1, space="PSUM") as psT, \
             tc.tile_pool(name="psDg", bufs=1, space="PSUM") as psDg, \
             tc.tile_pool(name="psP", bufs=1, space="PSUM") as psP:

            # ---------------- resident tiles
            ident = cpool.tile([P, P], f32)
            make_identity(nc, ident[:])
            identb = cpool.tile([P, P], bf16)
            nc.vector.tensor_copy(identb[:], ident[:])

            def load_bf16(name_tag, dram_ap, rows, cols):
                t = cpool.tile([rows, cols], bf16, name="cst_" + name_tag)
                s = stgpool.tile([rows, cols], f32, name="stg_" + name_tag,
                                 tag="cvt" + name_tag, bufs=1)
                nc.sync.dma_start(s[:], dram_ap)
                nc.vector.tensor_copy(t[:], s[:])
                return t

            idxLO_t = cpool.tile([P, max(C_LO, 1) * 8], mybir.dt.int16)
            nc.sync.dma_start(idxLO_t[:], idxLO[:])
            idxHI_t = cpool.tile([P, max(C_HI, 1) * 8], mybir.dt.int16)
            nc.sync.dma_start(idxHI_t[:], idxHI[:])
            dl_t = load_bf16("dl", dlA[:], P, C_CH)
            mh_t = load_bf16("mh", mhA[:], P, C_CH)
            mt_t = load_bf16("mt", mtA[:], P, C_CH)

            iota32_i = cpool.tile([P, SBATCH * P], mybir.dt.int32)
            nc.gpsimd.iota(iota32_i[:].rearrange("p (k d) -> p k d", d=P),
                           pattern=[[0, SBATCH], [1, P]], base=0,
                           channel_multiplier=0)
            iota32 = cpool.tile([P, SBATCH * P], bf16)
            nc.vector.tensor_copy(iota32[:], iota32_i[:])
            iotaWG_i = cpool.tile([P, W_G], mybir.dt.int32)
            nc.gpsimd.iota(iotaWG_i[:], pattern=[[1, W_G]], base=0,
                           channel_multiplier=0)
            iotaWG = cpool.tile([P, W_G], bf16)
            nc.vector.tensor_copy(iotaWG[:], iotaWG_i[:])
            ones_col = cpool.tile([1, P], bf16)
            nc.vector.memset(ones_col[:], 1.0)
            onesP = cpool.tile([P, 2], bf16)
            nc.vector.memset(onesP[:], 1.0)
            zero_t = cpool.tile([P, P], f32)
            nc.vector.memset(zero_t[:], 0.0)
            hT = cpool.tile([P, S_core], bf16)
            rdeg = cpool.tile([P, 2 * TILES], f32)
            bl_t = load_bf16("bl", batchloc[:], P, TILES)
            mrow_t = cpool.tile([W_G, GROUPS], f32)
            nc.sync.dma_start(mrow_t[:], maskrow[:])
            mrow_b = cpool.tile([W_G, GROUPS], bf16)
            nc.vector.tensor_copy(mrow_b[:], mrow_t[:])
            nc.sync.dma_start(mrow_d[:], mrow_b[:])
            gidx_t = cpool.tile([W_G, 1], mybir.dt.int32)
            nc.sync.dma_start(gidx_t[:], gidx[:])
            rcg_t = cpool.tile([P, 2], f32)
            nc.sync.dma_start(
                rcg_t[:], rcount_g[:2 * P, :].rearrange("(a b) o -> b (a o)", a=2))
            wAll_f = cpool.tile([P, 9 * F2], f32)
            for l in range(N_LAYERS):
                nc.sync.dma_start(wAll_f[:, (3 * l + 0) * F2:(3 * l + 1) * F2], w2cat[l])
                nc.sync.dma_start(wAll_f[:, (3 * l + 1) * F2:(3 * l + 2) * F2], w1cat[l])
                nc.sync.dma_start(wAll_f[:, (3 * l + 2) * F2:(3 * l + 3) * F2], w0cat[l])
            wAll = cpool.tile([P, 9 * F2], bf16)
            nc.vector.tensor_copy(wAll[:], wAll_f[:])
            bAll_f = cpool.tile([1, N_LAYERS * F2], f32)
            for l in range(N_LAYERS):
                nc.sync.dma_start(bAll_f[:, l * F2:(l + 1) * F2], bcat[l])
            bAll = cpool.tile([1, N_LAYERS * F2], bf16)
            nc.vector.tensor_copy(bAll[:], bAll_f[:])
            l1w = cpool.tile([P, 2 * F2], f32)  # two K-halves side by side
            nc.sync.dma_start(l1w[:, 0:F2], lin1w[0:P, :])
            nc.sync.dma_start(l1w[:, F2:2 * F2], lin1w[P:2 * P, :])
            l2w = cpool.tile([2 * H_HID, H_HID], f32)
            nc.sync.dma_start(l2w[:], lin2w[:])
            l2b = cpool.tile([H_HID, 1], f32)
            nc.sync.dma_start(l2b[:], lin2b[:])
            l1b = cpool.tile([2 * H_HID, 1], f32)
            nc.sync.dma_start(l1b[:], lin1b[:])
            l3w = cpool.tile([H_HID, N_CLS], f32)
            nc.sync.dma_start(l3w[:], lin3w[:])
            l3b = cpool.tile([N_CLS, 1], f32)
            nc.sync.dma_start(l3b[:], lin3b[:])
            spool_t = cpool.tile([P, TILES * W_G], bf16)
            gmax12 = cpool.tile([P, W_G], f32)
            nc.vector.memset(gmax12[:], 0.0)

            def wslice(l, which):  # 0=w2, 1=w1, 2=w0
                o = (3 * l + which) * F2
                return wAll[:, o:o + F2]

            def build_S(b):
                c0 = b * SBATCH
                nch = min(SBATCH, C_CH - c0)
                eq = spool.tile([P, SBATCH * P], bf16, tag="eq")
                sh = spool.tile([P, SBATCH * P], bf16, tag="sh")
                stt = spool.tile([P, SBATCH * P], bf16, tag="st")
                r3 = lambda ap: ap.rearrange("p (k d) -> p k d", d=P)[:, :nch, :]
                nc.vector.tensor_tensor(
                    out=r3(eq[:]),
                    in0=dl_t[:, c0:c0 + nch, None].to_broadcast([P, nch, P]),
                    in1=r3(iota32[:]), op=mybir.AluOpType.is_equal)
                nc.vector.tensor_tensor(
                    out=r3(sh[:]), in0=r3(eq[:]),
                    in1=mh_t[:, c0:c0 + nch, None].to_broadcast([P, nch, P]),
                    op=mybir.AluOpType.mult)
                nc.vector.tensor_tensor(
                    out=r3(stt[:]), in0=r3(eq[:]),
                    in1=mt_t[:, c0:c0 + nch, None].to_broadcast([P, nch, P]),
                    op=mybir.AluOpType.mult)
                return sh, stt

            # ---------------- x -> hT (feature-major)
            for t in range(TILES):
                xs = stgpool.tile([P, P], f32, tag="xs")
                nc.sync.dma_start(xs[:], x_own[t * P:(t + 1) * P, :])
                pt = psC.tile([P, 2 * P], f32, tag="c")
                nc.tensor.transpose(pt[:, 0:P], xs[:], ident[:])
                nc.scalar.copy(hT[:, t * P:(t + 1) * P], pt[:, 0:P])

            # ---------------- S_pool (one-hot local-graph selectors)
            for t in range(TILES):
                nc.vector.tensor_tensor(
                    out=spool_t[:, t * W_G:(t + 1) * W_G],
                    in0=bl_t[:, t:t + 1].to_broadcast([P, W_G]),
                    in1=iotaWG[:], op=mybir.AluOpType.is_equal)

            # ---------------- zero-fill pooled buffers
            for buf in (maxbuf, sumbuf):
                r = 0
                while r < NGP:
                    k = min(P, NGP - r)
                    nc.sync.dma_start(buf[r:r + k, :], zero_t[:k, :])
                    r += k

            pool_ps = psP.tile([W_G, F_IN], f32)

            def produce(l, dest):
                for t in range(TILES):
                    pc = psC.tile([P, 2 * P], f32, tag="c")
                    nc.tensor.matmul(pc[:, 0:F2], lhsT=hT[:, t * P:(t + 1) * P],
                                     rhs=wslice(l, 0), start=True, stop=True)
                    cs = stgpool.tile([P, F2], bf16, tag="cs")
                    nc.scalar.copy(cs[:], pc[:, 0:F2])
                    nc.sync.dma_start(dest[t * P:(t + 1) * P, :], cs[:])

            qglobal = [0]
            LANE2Q = [0, 0, 1, 1, 2, 2, 3, 3]

            def prop_pass(l, table, mode, do_pool, produce_next=None):
                # two gather streams: LO = table rows [0, HALF), HI = rest
                qctr = qglobal
                S_cache = [{}, {}]
                G_cache = [{}, {}]
                streams = ((idxLO_t, C_LO, table[0:HALF, :]),
                           (idxHI_t, C_HI, table[HALF:TOT, :]))

                def get_S(stream, b):
                    cache = S_cache[stream]
                    if b not in cache:
                        cache[b] = build_S(b)
                        for k in [k for k in cache if k < b - 1]:
                            del cache[k]
                    return cache[b]

                def get_G(stream, s):
                    cache = G_cache[stream]
                    if s not in cache:
                        idxt, C_S, tbl = streams[stream]
                        c0 = s * KG
                        nch = min(KG, C_S - c0)
                        gb = gpool.tile([P, KG * F_IN], bf16, tag=f"g{stream}")
                        nc.gpsimd.dma_gather(
                            gb[:, 0:nch * F_IN].rearrange(
                                "p (b f) -> p b f", f=F_IN),
                            tbl, idxt[:, c0 * 8:(c0 + nch) * 8],
                            nch * P, nch * P, F_IN,
                            queue_num=LANE2Q[qctr[0] % 8],
                            single_packet=False)
                        qctr[0] += 1
                        for k in [k for k in cache if k < s - 3]:
                            del cache[k]
                        cache[s] = gb
                    return cache[s]

                deg_pass = (l == 0 and mode == "A")

                for t in range(TILES):
                    stg = stgpool.tile([P, P], bf16, tag="hstg")
                    ps_t = psA.tile([P, F_IN], f32, tag="t")
                    if deg_pass:
                        pd = psDg.tile([P, 2], f32, tag="d")
                    chunks = ([(0, c) for c in
                               range(cogLO[t], cogLO[t] + cpgLO[t])] +
                              [(1, c) for c in
                               range(cogHI[t], cogHI[t] + cpgHI[t])])
                    n = len(chunks)
                    # hom sweep (psum chains must not interleave in-bank)
                    for i, (sm, sc) in enumerate(chunks):
                        gcol = sc if sm == 0 else C_LO + sc
                        gt = get_G(sm, sc // KG)[
                            :, (sc % KG) * F_IN:(sc % KG + 1) * F_IN]
                        sh, _ = get_S(sm, gcol // SBATCH)
                        jj = gcol % SBATCH
                        nc.tensor.matmul(
                            ps_t[:, 0:H_HID],
                            lhsT=sh[:, jj * P:(jj + 1) * P],
                            rhs=gt[:, 0:H_HID], start=i == 0,
                            stop=i == n - 1)
                        if deg_pass:
                            nc.tensor.matmul(
                                pd[:, 0:1],
                                lhsT=sh[:, jj * P:(jj + 1) * P],
                                rhs=onesP[:, 0:1], start=i == 0,
                                stop=i == n - 1)
                    # het sweep
                    for i, (sm, sc) in enumerate(chunks):
                        gcol = sc if sm == 0 else C_LO + sc
                        gt = G_cache[sm][sc // KG][
                            :, (sc % KG) * F_IN:(sc % KG + 1) * F_IN]
                        _, stt = get_S(sm, gcol // SBATCH)
                        jj = gcol % SBATCH
                        nc.tensor.matmul(
                            ps_t[:, H_HID:F_IN],
                            lhsT=stt[:, jj * P:(jj + 1) * P],
                            rhs=gt[:, H_HID:F_IN], start=i == 0,
                            stop=i == n - 1)
                        if deg_pass:
                            nc.tensor.matmul(
                                pd[:, 1:2],
                                lhsT=stt[:, jj * P:(jj + 1) * P],
                                rhs=onesP[:, 1:2], start=i == 0,
                                stop=i == n - 1)
                    if deg_pass:
                        nc.vector.tensor_scalar_max(
                            rdeg[:, 2 * t:2 * t + 2], pd[:], 1.0)
                        nc.vector.reciprocal(
                            rdeg[:, 2 * t:2 * t + 2],
                            rdeg[:, 2 * t:2 * t + 2])
                    ps_a = psB.tile([P, F_IN], f32, tag="a")
                    nc.tensor.matmul(
                        ps_a[:], lhsT=hT[:, t * P:(t + 1) * P],
                        rhs=wslice(l, 1 if mode == "A" else 2),
                        start=True, stop=(mode == "A"))
                    if mode == "B":
                        nc.tensor.matmul(ps_a[:], lhsT=ones_col[:, 0:P],
                                         rhs=bAll[:, l * F2:(l + 1) * F2],
                                         start=False, stop=True)
                    tp = smpool.tile([P, F_IN], f32, tag="tp")
                    nc.scalar.mul(tp[:, 0:H_HID], ps_t[:, 0:H_HID],
                                  rdeg[:, 2 * t:2 * t + 1])
                    nc.scalar.mul(tp[:, H_HID:F_IN], ps_t[:, H_HID:F_IN],
                                  rdeg[:, 2 * t + 1:2 * t + 2])
                    nc.vector.tensor_tensor(out=stg[:], in0=tp[:],
                                            in1=ps_a[:],
                                            op=mybir.AluOpType.add)
                    if mode == "B":
                        nc.vector.tensor_scalar_max(stg[:], stg[:], 0.0)
                    if mode == "A":
                        nc.sync.dma_start(u_own[t * P:(t + 1) * P, :], stg[:])
                    else:
                        ptr = psT.tile([P, P], bf16)
                        nc.tensor.transpose(ptr[:], stg[:], identb[:])
                        nc.scalar.copy(hT[:, t * P:(t + 1) * P], ptr[:])
                        if produce_next is not None:
                            # pipeline next layer's c = h @ W2 per tile so the
                            # produce+AllGather gap shrinks to just the AG
                            pc = psC.tile([P, 2 * P], f32, tag="c")
                            nc.tensor.matmul(pc[:, 0:F2],
                                             lhsT=hT[:, t * P:(t + 1) * P],
                                             rhs=wslice(produce_next, 0),
                                             start=True, stop=True)
                            cs = stgpool.tile([P, F2], bf16, tag="cs")
                            nc.scalar.copy(cs[:], pc[:, 0:F2])
                            nc.sync.dma_start(c_own[t * P:(t + 1) * P, :],
                                              cs[:])
                        if do_pool:
                            nc.tensor.matmul(
                                pool_ps[:],
                                lhsT=spool_t[:, t * W_G:(t + 1) * W_G],
                                rhs=stg[:],
                                start=(l == 1 and t == 0),
                                stop=(l == BISECT_LAYERS - 1 and t == TILES - 1))

            def max_pool_layer():
                m1 = smpool.tile([P, GROUPS], f32, tag="m1")
                nc.vector.tensor_reduce(
                    out=m1[:], in_=hT[:].rearrange("p (g d) -> p g d", d=DGRP),
                    axis=mybir.AxisListType.X, op=mybir.AluOpType.max)
                for li in range(W_G):
                    mrow_row = smpool.tile([1, GROUPS], bf16, tag="mrow")
                    nc.sync.dma_start(mrow_row[:], mrow_d[li:li + 1, :])
                    mb = psC.tile([P, 2 * P], f32, tag="c")
                    nc.tensor.matmul(mb[:, 0:GROUPS], lhsT=ones_col[:],
                                     rhs=mrow_row[:], start=True,
                                     stop=True)
                    msel = smpool.tile([P, GROUPS], f32, tag="msel")
                    nc.vector.tensor_tensor(out=msel[:], in0=m1[:],
                                            in1=mb[:, 0:GROUPS],
                                            op=mybir.AluOpType.mult)
                    gm = smpool.tile([P, 1], f32, tag="gm")
                    nc.vector.tensor_reduce(out=gm[:], in_=msel[:],
                                            axis=mybir.AxisListType.X,
                                            op=mybir.AluOpType.max)
                    nc.vector.tensor_tensor(out=gmax12[:, li:li + 1],
                                            in0=gmax12[:, li:li + 1], in1=gm[:],
                                            op=mybir.AluOpType.add)

            # ================ main layer loop
            produce(0, c_own)
            for l in range(BISECT_LAYERS):
                nc.gpsimd.collective_compute(
                    "AllGather", mybir.AluOpType.bypass,
                    ins=[c_own[:]], outs=[c_full[:]],
                    replica_groups=[list(range(NCORES))])
                prop_pass(l, c_full, "A", False)
                nc.gpsimd.collective_compute(
                    "AllGather", mybir.AluOpType.bypass,
                    ins=[u_own[:]], outs=[u_full[:]],
                    replica_groups=[list(range(NCORES))])
                prop_pass(l, u_full, "B", l >= 1,
                          produce_next=l + 1 if l + 1 < BISECT_LAYERS else None)
                if l >= 1:
                    max_pool_layer()

            # ================ pooled outputs -> DRAM -> AllReduce
            sums = smpool.tile([W_G, F_IN], f32, tag="sums")
            if BISECT_LAYERS >= 2:
                nc.scalar.copy(sums[:], pool_ps[:])
            else:
                nc.vector.memset(sums[:], 0.0)
            if not BISECT_NO_IDMA:
                nc.gpsimd.indirect_dma_start(
                    out=sumbuf[:],
                    out_offset=bass.IndirectOffsetOnAxis(ap=gidx_t[:, 0:1], axis=0),
                    in_=sums[:], in_offset=None)
            pmx = psC.tile([P, 2 * P], f32, tag="c")
            nc.tensor.transpose(pmx[0:W_G, 0:P], gmax12[:], ident[:])
            mxs = smpool.tile([W_G, P], f32, tag="mxs")
            nc.scalar.copy(mxs[:], pmx[0:W_G, 0:P])
            if not BISECT_NO_IDMA:
                nc.gpsimd.indirect_dma_start(
                    out=maxbuf[:],
                    out_offset=bass.IndirectOffsetOnAxis(ap=gidx_t[:, 0:1], axis=0),
                    in_=mxs[:], in_offset=None)
            nc.gpsimd.collective_compute(
                "AllReduce", mybir.AluOpType.max,
                ins=[maxbuf[:]], outs=[maxbuf_o[:]],
                replica_groups=[list(range(NCORES))])
            nc.gpsimd.collective_compute(
                "AllReduce", mybir.AluOpType.add,
                ins=[sumbuf[:]], outs=[sumbuf_o[:]],
                replica_groups=[list(range(NCORES))])

            # ================ head (replicated)
            rT = smpool.tile([P, 4 * P], f32, tag="rT")  # [feat128, max256|mean256]
            for half in range(2):
                mx = smpool.tile([P, F_IN], f32, tag="mx")
                nc.sync.dma_start(mx[:], maxbuf_o[half * P:(half + 1) * P, :])
                sm = smpool.tile([P, F_IN], f32, tag="smh")
                nc.sync.dma_start(sm[:], sumbuf_o[half * P:(half + 1) * P, :])
                nc.vector.tensor_scalar(out=sm[:], in0=sm[:],
                                        scalar1=rcg_t[:, half:half + 1],
                                        scalar2=None, op0=mybir.AluOpType.mult)
                pmxT = psC.tile([P, 2 * P], f32, tag="c")
                nc.tensor.transpose(pmxT[:, 0:P], mx[:], ident[:])
                nc.scalar.copy(rT[:, half * P:(half + 1) * P], pmxT[:, 0:P])
                psmT = psC.tile([P, 2 * P], f32, tag="c")
                nc.tensor.transpose(psmT[:, 0:P], sm[:], ident[:])
                nc.scalar.copy(rT[:, 2 * P + half * P:2 * P + (half + 1) * P],
                               psmT[:, 0:P])

            z1p = psC.tile([P, 2 * P], f32, tag="c")
            nc.tensor.matmul(z1p[:F2, 0:2 * P], lhsT=l1w[:, 0:F2],
                             rhs=rT[:, 0:2 * P], start=True, stop=False)
            nc.tensor.matmul(z1p[:F2, 0:2 * P], lhsT=l1w[:, F2:2 * F2],
                             rhs=rT[:, 2 * P:4 * P], start=False, stop=True)
            z1 = smpool.tile([F2, 2 * P], f32, tag="z1")
            nc.scalar.activation(z1[:], z1p[:F2, 0:2 * P],
                                 mybir.ActivationFunctionType.Relu,
                                 bias=l1b[:, 0:1], scale=1.0)
            z2p = psC.tile([P, 2 * P], f32, tag="c")
            nc.tensor.matmul(z2p[:H_HID, 0:2 * P], lhsT=l2w[:], rhs=z1[:],
                             start=True, stop=True)
            z2 = smpool.tile([H_HID, 2 * P], f32, tag="z2")
            nc.scalar.activation(z2[:], z2p[:H_HID, 0:2 * P],
                                 mybir.ActivationFunctionType.Relu,
                                 bias=l2b[:, 0:1], scale=1.0)
            z3p = psC.tile([P, 2 * P], f32, tag="c")
            nc.tensor.matmul(z3p[:N_CLS, 0:2 * P], lhsT=l3w[:], rhs=z2[:],
                             start=True, stop=True)
            z3 = smpool.tile([N_CLS, 2 * P], f32, tag="z3")
            nc.scalar.activation(z3[:], z3p[:N_CLS, 0:2 * P],
                                 mybir.ActivationFunctionType.Identity,
                                 bias=l3b[:, 0:1], scale=1.0)
            for half in range(2):
                lg = psC.tile([P, 2 * P], f32, tag="c")
                nc.tensor.transpose(lg[:, 0:N_CLS],
                                    z3[:, half * P:(half + 1) * P],
                                    ident[0:N_CLS, 0:N_CLS])
                lgs = smpool.tile([P, N_CLS], f32, tag="lgs")
                nc.vector.tensor_copy(lgs[:], lg[:, 0:N_CLS])
                rmax = smpool.tile([P, 1], f32, tag="rmax")
                nc.vector.tensor_reduce(out=rmax[:], in_=lgs[:],
                                        axis=mybir.AxisListType.X,
                                        op=mybir.AluOpType.max)
                xm = smpool.tile([P, N_CLS], f32, tag="xm")
                nc.vector.tensor_scalar(out=xm[:], in0=lgs[:],
                                        scalar1=rmax[:, 0:1], scalar2=None,
                                        op0=mybir.AluOpType.subtract)
                ex = smpool.tile([P, N_CLS], f32, tag="ex")
                nc.scalar.activation(ex[:], xm[:],
                                     mybir.ActivationFunctionType.Exp)
                sume = smpool.tile([P, 1], f32, tag="sume")
                nc.vector.tensor_reduce(out=sume[:], in_=ex[:],
                                        axis=mybir.AxisListType.X,
                                        op=mybir.AluOpType.add)
                lse = smpool.tile([P, 1], f32, tag="lse")
                nc.scalar.activation(lse[:], sume[:],
                                     mybir.ActivationFunctionType.Ln)
                res = smpool.tile([P, N_CLS], f32, tag="res")
                nc.vector.tensor_scalar(out=res[:], in0=xm[:],
                                        scalar1=lse[:, 0:1], scalar2=None,
                                        op0=mybir.AluOpType.subtract)
                nc.sync.dma_start(out[half * P:(half + 1) * P, :], res[:])

    nc.compile()
    return nc


# ------------------------------------------------------------------ runner
def _make_runner(nc, n_cores):
    import jax
    import concourse.mybir as mybir
    from jax.experimental.shard_map import shard_map
    from jax.sharding import Mesh, NamedSharding, PartitionSpec
    from concourse.bass2jax import (_bass_exec_p, install_neuronx_cc_hook,
                                    partition_id_tensor)

    install_neuronx_cc_hook()
    partition_name = nc.partition_id_tensor.name if nc.partition_id_tensor else None
    in_names, out_names, out_avals = [], [], []
    for alloc in nc.m.functions[0].allocations:
        if not isinstance(alloc, mybir.MemoryLocationSet):
            continue
        name = alloc.memorylocations[0].name
        if alloc.kind == "ExternalInput":
            if name != partition_name:
                in_names.append(name)
        elif alloc.kind == "ExternalOutput":
            out_names.append(name)
            out_avals.append(jax.core.ShapedArray(
                tuple(alloc.tensor_shape), mybir.dt.np(alloc.dtype)))
    n_params = len(in_names)
    all_in = list(in_names) + list(out_names)
    if partition_name is not None:
        all_in.append(partition_name)

    def _body(*args):
        operands = list(args)
        if partition_name is not None:
            operands.append(partition_id_tensor())
        return tuple(_bass_exec_p.bind(
            *operands, out_avals=tuple(out_avals), in_names=tuple(all_in),
            out_names=tuple(out_names), lowering_input_output_aliases=(),
            sim_require_finite=False, sim_require_nnan=False, nc=nc))

    devices = jax.devices()[:n_cores]
    mesh = Mesh(np.asarray(devices), ("core",))
    nin = n_params + len(out_names)
    sharded = jax.jit(shard_map(
        _body, mesh=mesh, in_specs=(PartitionSpec("core"),) * nin,
        out_specs=(PartitionSpec("core"),) * len(out_names), check_rep=False),
        keep_unused=True)
    sharding = NamedSharding(mesh, PartitionSpec("core"))

    def stage(in_maps):
        import jax as _jax
        concat_in = [np.concatenate([np.asarray(in_maps[c][nm])
                                     for c in range(n_cores)], axis=0)
                     for nm in in_names]
        concat_zero = [np.zeros((n_cores * a.shape[0], *a.shape[1:]), a.dtype)
                       for a in out_avals]
        return [_jax.device_put(x, sharding) for x in concat_in + concat_zero]

    def call(staged):
        import jax as _jax
        outs = sharded(*staged)
        _jax.block_until_ready(outs)
        return outs

    call.sharded = sharded

    def fetch(outs):
        return [{nm: np.asarray(outs[i]).reshape(n_cores, *out_avals[i].shape)[c]
                 for i, nm in enumerate(out_names)} for c in range(n_cores)]

    return stage, call, fetch


_CACHE = {}


def _get_compiled(st):
    key = (st["S_core"], st["C_LO"], st["C_HI"],
           tuple(st["cpgLO"].tolist()), tuple(st["cpgHI"].tolist()))
    if key not in _CACHE:
        nc = _build(st)
        _CACHE[key] = (nc, _make_runner(nc, NCORES))
    return _CACHE[key]


def _in_maps(st, per_core, x, inputs):
    node_slot = st["node_slot"]
    S_core = st["S_core"]
    hom_W = np.asarray(inputs["hom_W"], np.float32)
    het_W = np.asarray(inputs["het_W"], np.float32)
    hom_b = np.asarray(inputs["hom_b"], np.float32)
    het_b = np.asarray(inputs["het_b"], np.float32)
    w2 = np.ascontiguousarray(np.concatenate([hom_W[:, 2], het_W[:, 2]], axis=2))
    w1 = np.ascontiguousarray(np.concatenate([hom_W[:, 1], het_W[:, 1]], axis=2))
    w0 = np.ascontiguousarray(np.concatenate([hom_W[:, 0], het_W[:, 0]], axis=2))
    bb = np.ascontiguousarray(np.concatenate([hom_b, het_b], axis=1)[:, None, :])
    rcount = np.zeros((NGP, 1), np.float32)
    rcount[:N_GRAPHS, 0] = 1.0 / np.maximum(st["counts"], 1.0)

    x = np.asarray(x, np.float32)
    maps = []
    for c in range(NCORES):
        xo = np.zeros((S_core, F_IN), np.float32)
        m = (node_slot >= c * S_core) & (node_slot < (c + 1) * S_core)
        xo[node_slot[m] - c * S_core] = x[m]
        pc = per_core[c]
        maps.append({
            "x_own": xo, "idxLO": pc["idxLO"], "idxHI": pc["idxHI"],
            "dlA": pc["dlA"], "mhA": pc["mhA"], "mtA": pc["mtA"],
            "batchloc": pc["batchloc"],
            "maskrow": pc["maskrow"], "gidx": pc["gidx"], "rcount_g": rcount,
            "w2cat": w2, "w1cat": w1, "w0cat": w0, "bcat": bb,
            "lin1w": np.asarray(inputs["lin1_W"], np.float32),
            "lin1b": np.asarray(inputs["lin1_b"], np.float32)[:, None],
            "lin2w": np.asarray(inputs["lin2_W"], np.float32),
            "lin2b": np.asarray(inputs["lin2_b"], np.float32)[:, None],
            "lin3w": np.asarray(inputs["lin3_W"], np.float32),
            "lin3b": np.asarray(inputs["lin3_b"], np.float32)[:, None]})
    return maps


def kernel(**inputs):
    x = np.asarray(inputs["x"])
    edge_index = np.asarray(inputs["edge_index"])
    batch = np.asarray(inputs["batch"])
    st, per_core = _prep(edge_index, batch, inputs["hom_mask"], inputs["het_mask"])
    nc, (stage, call, fetch) = _get_compiled(st)
    maps = _in_maps(st, per_core, x, inputs)
    staged = stage(maps)
    outs = call(staged)
    return fetch(outs)[0]["out"].astype(np.float32)
